# revision 23
# baseline (speedup 1.0000x reference)
"""Trainium2 Bass kernel for nn_Model_39676907883957 (dense_transformer).

Math (per batch element b, with S = D = N = 2048):
    q = Xq @ Wq^T + bq            # [S, D]
    kT = Wk @ Xk^T + bk[:, None]  # [D, S]  (k projected directly in transposed layout)
    v = Xv @ Wv^T + bv            # [S, D]
    scores[i, j] = sum_m q[m, i] * kT[m, j]          # q^T @ k^T
    attn = softmax_rows(scores)
    outT[i, n] = sum_j attn[i, j] * v[j, n]          # host transposes -> out[n, i]

Sharding: data-parallel over batch, B=8 -> one batch element per NeuronCore.

Numerics: all five matmuls run single-pass fp16 on the PE (fp32 PSUM
accumulate).  The softmax logits have std ~45 but the row max-vs-2nd
gap is ~9, so the ~0.03 logit noise from fp16 inputs is harmless:
simulated end-to-end rel err ~3.7e-3 (gate 2e-2).

Dataflow: kT and v are projected directly into SBUF-resident tiles.  The
three projections run chunk-outer over four 512-wide output chunks, with
the moving operand held as four [128, 8192] chunk-tiles in one shared
pool: each phase's chunk-c tile is recycled as soon as the previous
phase retires its chunk c (25% into the phase), so cross-phase resident
loads are never gated on a full phase and each loads with a single DMA.
Stationary operands stream from host-preblocked layouts (contiguous
8KB/partition descriptors).

The scores/softmax/attn@v stage is a software-pipelined loop over the 16
query row-blocks: scores(i) -> softmax(i) -> xbar-transpose of the attn
row-block into 16 [j, i] staging tiles (split across both HWDGE queues)
-> attn@v matmuls lagging two blocks so the PE never waits.  PSUM's 8
banks double-buffer exactly (4 scores + 4 attn@v tiles per iteration).
Output is written transposed; the host transposes back.
"""

import numpy as np

import concourse.bass as bass
import concourse.bacc as bacc
import concourse.tile as tile
import concourse.mybir as mybir
from concourse.bass_utils import run_bass_kernel_spmd

B, S, D = 8, 2048, 2048
N = 2048                 # S == D
KT = N // 128            # 16 contraction tiles
NCHUNK = N // 512        # 4 free-dim chunks of 512
F16 = mybir.dt.float16
F32 = mybir.dt.float32
AX = mybir.AxisListType.X
EXP = mybir.ActivationFunctionType.Exp

_compiled = {}


def _build():
    nc = bacc.Bacc("TRN2", target_bir_lowering=False, debug=False)

    # ExternalInputs (per core). x* are host-transposed activations [d, s],
    # w* are host-transposed weights W^T [d, e]; all fp16.
    # xq/xv/wk additionally use the host-preblocked layout (_block_layout).
    xq = nc.dram_tensor("xq", [N, N], F16, kind="ExternalInput").ap()
    xk = nc.dram_tensor("xk", [N, N], F16, kind="ExternalInput").ap()
    xv = nc.dram_tensor("xv", [N, N], F16, kind="ExternalInput").ap()
    wq = nc.dram_tensor("wq", [N, N], F16, kind="ExternalInput").ap()
    wk = nc.dram_tensor("wk", [N, N], F16, kind="ExternalInput").ap()
    wv = nc.dram_tensor("wv", [N, N], F16, kind="ExternalInput").ap()
    # biases: bqb/bvb broadcast across partitions [128, N]; bkp partition-major [128, KT]
    bqb = nc.dram_tensor("bqb", [128, N], F16, kind="ExternalInput").ap()
    bkp = nc.dram_tensor("bkp", [128, KT], F32, kind="ExternalInput").ap()
    bvb = nc.dram_tensor("bvb", [128, N], F16, kind="ExternalInput").ap()

    out = nc.dram_tensor("out", [N, N], F32, kind="ExternalOutput").ap()

    with tile.TileContext(nc, pool_alloc_mode="queue") as tc:
        with tc.tile_pool(name="dram", bufs=1, space="DRAM") as dram:
            q_f = dram.tile([N, N], F16, tag="q_f")

            with tc.tile_pool(name="psum", bufs=8, space="PSUM") as psum:
                with tc.tile_pool(name="kres", bufs=1) as krespool, \
                     tc.tile_pool(name="vres", bufs=1) as vrespool:
                    kres = [krespool.tile([128, N], F16, tag=f"kr{t}",
                                          name=f"kr{t}") for t in range(KT)]
                    vres = [vrespool.tile([128, N], F16, tag=f"vr{t}",
                                          name=f"vr{t}") for t in range(KT)]
                    with tc.tile_pool(name="stream", bufs=1) as stream:
                        _proj(nc, tc, psum, stream, xq, wq, bqb, "q",
                              out_dram=q_f)
                        _proj_cols(nc, tc, psum, stream, xk, wk, bkp, kres)
                        _proj(nc, tc, psum, stream, xv, wv, bvb, "v",
                              out_res=vres)
                    _attention(nc, tc, psum, q_f, kres, vres, out)

    nc.compile()
    return nc


def _load_kblock_blocked(nc, pool, dram_ap, col_blk, tag):
    """Load logical col-block `col_blk` of a host-preblocked [d, s] tensor
    (see _block_layout) as one [128, N] tile: [:, k*128:(k+1)*128] is
    contraction-tile k.  Contiguous row-block read -> 8KB/partition DMA."""
    t = pool.tile([128, N], F16, tag=tag)
    nc.scalar.dma_start(t[:], dram_ap[col_blk * 128:(col_blk + 1) * 128, :])
    return t


def _load_chunk_tiles(nc, stream, src):
    """Load an [N, N] moving operand as NCHUNK [128, KT*512] chunk-tiles from
    the shared stream pool, one DMA each:
    tile[c][p, k*512 + e'] = src[k*128 + p, c*512 + e']."""
    ts = []
    for c in range(NCHUNK):
        t = stream.tile([128, KT * 512], F16, tag=f"c{c}")
        s3 = src[:, c * 512:(c + 1) * 512].rearrange("(k p) e -> p k e", p=128)
        nc.gpsimd.dma_start(t[:].rearrange("p (k e) -> p k e", k=KT), s3)
        ts.append(t)
    return ts


def _proj(nc, tc, psum, stream, x, w, bias_bcast, tag, out_dram=None,
          out_res=None):
    """q/v-style projection: out[s, e] = sum_d X^T[d, s] * W^T[d, e] + bias[e].
    Chunk-outer; stationary = preblocked activation col-blocks (re-streamed per
    chunk), moving = shared-pool weight chunk-tiles."""
    with (
        tc.tile_pool(name=f"pj_x{tag}", bufs=2) as xpool,
        tc.tile_pool(name=f"pj_s{tag}", bufs=2) as spool,
        tc.tile_pool(name=f"pj_b{tag}", bufs=1) as bpool,
    ):
        bb = bpool.tile([128, N], F16, tag="bias")
        nc.sync.dma_start(bb[:], bias_bcast[:])
        wcs = _load_chunk_tiles(nc, stream, w)
        for c in range(NCHUNK):
            cs = slice(c * 512, (c + 1) * 512)
            for s in range(KT):
                a = _load_kblock_blocked(nc, xpool, x, s, "a")
                ps = psum.tile([128, 512], F32)
                for k in range(KT):
                    nc.tensor.matmul(ps[:], a[:, k * 128:(k + 1) * 128],
                                     wcs[c][:, k * 512:(k + 1) * 512],
                                     start=(k == 0), stop=(k == KT - 1))
                if out_res is not None:
                    nc.vector.tensor_add(out_res[s][:, cs], ps[:], bb[:, cs])
                else:
                    h16 = spool.tile([128, 512], F16, tag="h16")
                    nc.vector.tensor_add(h16[:], ps[:], bb[:, cs])
                    nc.sync.dma_start(out_dram[s * 128:(s + 1) * 128, cs],
                                      h16[:])


def _proj_cols(nc, tc, psum, stream, x, w, bias_part, kres):
    """kT projection into resident SBUF tiles:
    kres[e][p, s] = sum_d W^T[d, 128e+p] * X^T[d, s] + bk[128e+p].
    Chunk-outer; stationary = preblocked weight col-blocks, moving =
    shared-pool activation chunk-tiles."""
    with (
        tc.tile_pool(name="pk_w", bufs=2) as wpool,
        tc.tile_pool(name="pk_b", bufs=1) as bpool,
    ):
        bp = bpool.tile([128, KT], F32, tag="biasp")
        nc.sync.dma_start(bp[:], bias_part[:])
        xcs = _load_chunk_tiles(nc, stream, x)
        for c in range(NCHUNK):
            cs = slice(c * 512, (c + 1) * 512)
            for e in range(KT):
                g = _load_kblock_blocked(nc, wpool, w, e, "g")
                ps = psum.tile([128, 512], F32)
                for k in range(KT):
                    nc.tensor.matmul(ps[:], g[:, k * 128:(k + 1) * 128],
                                     xcs[c][:, k * 512:(k + 1) * 512],
                                     start=(k == 0), stop=(k == KT - 1))
                nc.vector.tensor_scalar_add(kres[e][:, cs], ps[:],
                                            bp[:, e:e + 1])


def _attention(nc, tc, psum, q_f, kres, vres, out):
    """Fused scores -> softmax -> attn^T (xbar) -> attn@v, pipelined over the
    16 query row-blocks with the attn@v matmuls lagging two blocks behind."""
    with (
        tc.tile_pool(name="at_q", bufs=2) as qpool,
        tc.tile_pool(name="at_s", bufs=2) as spool,
        tc.tile_pool(name="at_t", bufs=4) as tpool,
        tc.tile_pool(name="at_a", bufs=3) as apool,
        tc.tile_pool(name="at_o", bufs=4) as opool,
    ):
        def load_q4(ib):
            """One DMA for 4 q col-blocks: tile[p, t*512 + di*128 + s'] =
            q[t*128+p, (4*ib+di)*128 + s']  (1KB/partition segments)."""
            t = qpool.tile([128, KT * 512], F16, tag="qq")
            src = q_f[:, ib * 512:(ib + 1) * 512].rearrange(
                "(t p) s -> p t s", p=128
            )
            nc.sync.dma_start(t[:].rearrange("p (t s) -> p t s", t=KT), src)
            return t

        def emit_av(i, atT):
            """attn@v for row-block i: outT[i-block, n] = sum_j attnT[j,:]*v[j,n]."""
            for c in range(NCHUNK):
                cs = slice(c * 512, (c + 1) * 512)
                ps = psum.tile([128, 512], F32)
                for j in range(KT):
                    nc.tensor.matmul(ps[:], atT[j][:, :], vres[j][:, cs],
                                     start=(j == 0), stop=(j == KT - 1))
                o32 = opool.tile([128, 512], F32, tag="o32")
                nc.vector.tensor_copy(o32[:], ps[:])
                nc.sync.dma_start(out[i * 128:(i + 1) * 128, cs], o32[:])

        qq = load_q4(0)
        hist = []                      # attn^T tile sets awaiting their attn@v
        for i in range(KT):
            di = i % 4
            # scores matmuls for block i
            pss = []
            for c in range(NCHUNK):
                cs = slice(c * 512, (c + 1) * 512)
                ps = psum.tile([128, 512], F32)
                for k in range(KT):
                    nc.tensor.matmul(
                        ps[:],
                        qq[:, k * 512 + di * 128:k * 512 + (di + 1) * 128],
                        kres[k][:, cs], start=(k == 0), stop=(k == KT - 1))
                pss.append(ps)
            # prefetch the next 4-block q tile two iterations early
            if di == 2 and i + 2 < KT:
                qq_next = load_q4((i + 2) // 4)
            elif di == 3:
                qq = qq_next
            # softmax over the full 2048-wide row
            m4 = tpool.tile([128, NCHUNK], F32, tag="m4")
            for c in range(NCHUNK):
                nc.vector.reduce_max(m4[:, c:c + 1], pss[c][:], axis=AX)
            mx = tpool.tile([128, 1], F32, tag="mx")
            nc.vector.reduce_max(mx[:], m4[:], axis=AX)
            negm = tpool.tile([128, 1], F32, tag="negm")
            nc.scalar.mul(negm[:], mx[:], -1.0)
            af32 = spool.tile([128, N], F32, tag="af32")
            sume = tpool.tile([128, NCHUNK], F32, tag="sume")
            for c in range(NCHUNK):
                cs = slice(c * 512, (c + 1) * 512)
                nc.scalar.activation(af32[:, cs], pss[c][:], EXP,
                                     bias=negm[:], scale=1.0,
                                     accum_out=sume[:, c:c + 1])
            tot = tpool.tile([128, 1], F32, tag="tot")
            nc.vector.reduce_sum(tot[:], sume[:], axis=AX)
            rcp = tpool.tile([128, 1], F32, tag="rcp")
            nc.vector.reciprocal(rcp[:], tot[:])
            a16 = spool.tile([128, N], F16, tag="a16")
            nc.vector.tensor_scalar_mul(a16[:], af32[:], rcp[:])
            # transpose attn row-block into 16 [j, i] staging tiles, split
            # across both HWDGE queues so neither serializes the pipeline
            atT = []
            for j in range(KT):
                t = apool.tile([128, 128], F16, tag=f"t{j}")
                eng = nc.sync if j % 2 == 0 else nc.scalar
                eng.dma_start_transpose(t[:], a16[:, j * 128:(j + 1) * 128])
                atT.append(t)
            hist.append(atT)
            # attn@v lags two blocks so the PE never waits on softmax+transpose
            if i >= 2:
                emit_av(i - 2, hist[i - 2])
        emit_av(KT - 2, hist[KT - 2])
        emit_av(KT - 1, hist[KT - 1])


def _block_layout(A16):
    """Permute a [d, s] fp16 matrix so that _load_kblock_blocked's row-block
    `blk` holds, at [p, k*128 + s'], the element A[k*128 + p, blk*128 + s']."""
    return np.ascontiguousarray(
        A16.reshape(KT, 128, KT, 128).transpose(2, 1, 0, 3)
    ).reshape(N, N)


def prepare_in_maps(query, key_, value, Wq, bq, Wk, bk, Wv, bv):
    query = np.asarray(query, dtype=np.float32)
    key_ = np.asarray(key_, dtype=np.float32)
    value = np.asarray(value, dtype=np.float32)
    Wq = np.asarray(Wq, dtype=np.float32)
    Wk = np.asarray(Wk, dtype=np.float32)
    Wv = np.asarray(Wv, dtype=np.float32)
    bq = np.asarray(bq, dtype=np.float32)
    bk = np.asarray(bk, dtype=np.float32)
    bv = np.asarray(bv, dtype=np.float32)

    wqT = np.ascontiguousarray(Wq.T).astype(np.float16)
    wkT = _block_layout(np.ascontiguousarray(Wk.T).astype(np.float16))
    wvT = np.ascontiguousarray(Wv.T).astype(np.float16)
    bqb = np.broadcast_to(bq, (128, N)).astype(np.float16)
    bvb = np.broadcast_to(bv, (128, N)).astype(np.float16)
    bkp = np.ascontiguousarray(bk.reshape(KT, 128).T)

    in_maps = []
    for b in range(B):
        in_maps.append({
            "xq": _block_layout(np.ascontiguousarray(query[b].T).astype(np.float16)),
            "xk": np.ascontiguousarray(key_[b].T).astype(np.float16),
            "xv": _block_layout(np.ascontiguousarray(value[b].T).astype(np.float16)),
            "wq": wqT, "wk": wkT, "wv": wvT,
            "bqb": bqb, "bkp": bkp, "bvb": bvb,
        })
    return in_maps


def get_nc():
    if "nc" not in _compiled:
        _compiled["nc"] = _build()
    return _compiled["nc"]


def kernel(query, key_, value, Wq, bq, Wk, bk, Wv, bv):
    in_maps = prepare_in_maps(query, key_, value, Wq, bq, Wk, bk, Wv, bv)
    res = run_bass_kernel_spmd(get_nc(), in_maps, core_ids=list(range(B)))
    # device computes outT = attn @ v; reference output is its transpose
    return np.stack([res.results[b]["out"].T for b in range(B)]).astype(np.float32)


if __name__ == "__main__":
    rng = np.random.default_rng(0)
    inputs = {
        "query": rng.standard_normal((B, S, D), dtype=np.float32),
        "key_": rng.standard_normal((B, S, D), dtype=np.float32),
        "value": rng.standard_normal((B, S, D), dtype=np.float32),
        "Wq": (rng.standard_normal((D, D), dtype=np.float32) / np.sqrt(D)),
        "bq": rng.standard_normal(D).astype(np.float32) * 0.01,
        "Wk": (rng.standard_normal((D, D), dtype=np.float32) / np.sqrt(D)),
        "bk": rng.standard_normal(D).astype(np.float32) * 0.01,
        "Wv": (rng.standard_normal((D, D), dtype=np.float32) / np.sqrt(D)),
        "bv": rng.standard_normal(D).astype(np.float32) * 0.01,
    }
    out = kernel(**inputs)
    print("out", out.shape, out.dtype)
